# revision 36
# baseline (speedup 1.0000x reference)
"""ContrastivePretrainedSAGE Trainium2 kernel v4 (8-core SPMD).

Design: nodes sharded by id range (12544 slots/core = 98 windows of 128).
Edges routed to the dst-owning core. Within each core, windows are
PERMUTED (sorted by incoming-edge count, descending) so that window-slot
j has a similar count on every core; the single SPMD program sizes each
(slot, src-group) run as max-over-cores ceil(cnt/16) 16-row units
(~218k gathered rows/core vs 250.9k for fixed 5-tile runs). Runs pack
back-to-back in the (superwindow, group) chunk stream with no alignment
constraints: the slot table PHASE-ENCODES the target window
(slot16 = 128*jl + dst_slot, fp16-exact, jl unique within a chunk), so a
window's mask - built by one is_equal against that window's iota band -
automatically zeroes every row belonging to other windows or padding.
Every mask matmul is then a full K=128, base-partition-0 matmul (the
only PE config that runs reliably), regardless of where runs start/end.

Source features are fetched with InstDMAGatherAnt from an fp8(e4m3) copy
of x (256B rows), 1024 rows per op, FOUR SWDGE queues so descriptor
generation runs concurrently (measured: 1q 8.8 ns/idx, 2q 4.6, 4q
3.2-3.6 - the Q7 descriptor generator is this kernel's bottleneck).
Mask matmuls accumulate into a [128,256] f32 PSUM region per window
(2 windows/bank); adjacent tile pairs of the same window fuse into one
fp8 DoubleRow matmul (K=256/pass, 2x PE rate).

Everything derivable from the small weights is folded on the host
(u=W_res.T@w_score, c=b_res.w_score+b_score, a=sigmoid(alpha), with
(1-a) pre-multiplied into w_score/u/c and a into reranker_scores), so
the per-window epilogue is only:
  DVE:  aggr = psum * invd          (1 instr)
  PE:   ph = x@[W_r.T | u'] + ones@[b_l | c'] + aggr@W_l.T  (via 2
        transposes + 5 matmuls)
  ACT:  hrelu = relu(ph[:,0:128]), 2 PSUM->SBUF transpose copies
  DVE:  out[:,j] = reduce_add(hrelu * ws', init=ph[:,128])  (1 instr)
and one final out += a*rer over all windows. deg/invd are host-side
(index-space bincount).
"""
from dataclasses import dataclass

import numpy as np
import ml_dtypes

import concourse.bass as bass
import concourse.mybir as mybir
import concourse.tile as tile
from concourse.bass_utils import run_bass_kernel_spmd

F32 = mybir.dt.float32
F16 = mybir.dt.float16
BF16 = mybir.dt.bfloat16
FP8 = mybir.dt.float8e4
I16 = mybir.dt.int16
AOP = mybir.AluOpType
ACT = mybir.ActivationFunctionType
NCORE = 8
PAD_SLOT = 1023.0
NP_FP8 = ml_dtypes.float8_e4m3
USE_FP8 = True
USE_TTR = False      # fuse h*ws + reduce + xu via tensor_tensor_reduce
USE_ACTCOPY = False  # PSUM->SBUF transpose copies on ACT instead of DVE
USE_BLC = True      # add [b_l | c] via ones-row matmul instead of DVE
ALIGN = 16          # run-length granularity in rows


def split_sync_waits(nc) -> int:
    n_split = 0
    for f in nc.m.functions:
        for bb in f.blocks:
            out = []
            changed = False
            for ins in bb.instructions:
                si = ins.sync_info
                waits = list(si.on_wait) if si is not None and si.on_wait else []
                if len(waits) > 1:
                    for g, w in enumerate(waits[:-1]):
                        nop = mybir.InstNoOp(name=f"{ins.name}-waitsplit-{g}")
                        nop.engine = ins.engine
                        nop.sync_info = mybir.SyncInfo(on_wait=[w], on_update=[])
                        out.append(nop)
                    si.on_wait = waits[-1:]
                    changed = True
                    n_split += 1
                out.append(ins)
            if changed:
                bb.instructions.clear()
                for i in out:
                    bb.instructions.append(i)
    return n_split


def finish(nc):
    split_sync_waits(nc)
    import bass_rust
    from concourse.library_config import all_libraries, standard
    m = {}
    for lib in all_libraries:
        for it in lib.instructions:
            m[it] = m.get(it, 0) | (1 << lib.index)
    bass_rust.insert_library_loads(nc, m, len(all_libraries), standard.index)
    mybir.codegen_inst_isa_subclasses(nc)
    return nc


@dataclass
class Cfg:
    nsw: int          # superwindows per core
    bw: int           # windows (slots) per superwindow
    nx: int           # padded gather-table rows
    gs: int           # group size (rows per source group, <= 32768)
    ngroups: int = 4
    d_in: int = 256
    d_h: int = 128
    max_op: int = 1024

    @property
    def wpc(self):
        return self.nsw * self.bw

    @property
    def npc(self):
        return self.wpc * 128


@dataclass
class Layout:
    """Unified (all-core) tile-stream layout, host-computed.

    ops: list of gather ops (t0, nt, g, wins) where wins is the list of
         (j, tloc, ntw, islast): window-slot j covers op-local tiles
         [tloc, tloc+ntw); islast marks the window's final op -> its
         epilogue runs after those matmuls.
    ntiles: stream length in tiles
    """
    ops: list
    ntiles: int


def wrap_idx(idx: np.ndarray) -> np.ndarray:
    """[L] -> [128, L/16] int16 wrapped (i at [i%16, i//16]), replicated 8x."""
    L = len(idx)
    assert L % 16 == 0
    block = np.zeros((16, L // 16), np.int16)
    block[np.arange(L) % 16, np.arange(L) // 16] = idx.astype(np.int16)
    return np.tile(block, (8, 1))


def preprocess(x, edge_index, reranker_scores, cfg: Cfg):
    """Index-space edge routing + pure layout prep of per-core inputs."""
    N = x.shape[0]
    src = np.asarray(edge_index[0], dtype=np.int64)
    dst = np.asarray(edge_index[1], dtype=np.int64)
    rer = np.asarray(reranker_scores, dtype=np.float32)

    x_pad = np.zeros((cfg.nx, cfg.d_in), np.float32)
    x_pad[:N] = np.asarray(x, dtype=np.float32)
    x_gt = np.ascontiguousarray(
        x_pad.astype(NP_FP8 if USE_FP8 else ml_dtypes.bfloat16))
    xT_bf = np.ascontiguousarray(x_pad.astype(ml_dtypes.bfloat16).T)

    npc, wpc, ng = cfg.npc, cfg.wpc, cfg.ngroups
    g_of = src // cfg.gs
    deg_full = np.bincount(dst, minlength=N)

    # per-core routing + window-to-slot assignment: start from
    # sort-by-count-desc, then refine with iterative linear assignment to
    # minimize sum over (slot, group) of max-over-cores ceil(cnt/ALIGN)
    perm = np.zeros((NCORE, wpc), np.int64)       # slot j -> orig window
    craw = np.zeros((NCORE, wpc, ng), np.int64)
    es, eg, ed, ew = [], [], [], []
    for c in range(NCORE):
        lo = c * npc
        m = (dst >= lo) & (dst < lo + npc)
        s_c = src[m]
        d_c = dst[m] - lo
        g_c = g_of[m]
        w_c = d_c >> 7
        craw[c] = np.bincount(w_c * ng + g_c, minlength=wpc * ng).reshape(wpc, ng)
        perm[c] = np.argsort(-craw[c].sum(1), kind="stable")
        es.append(s_c); ed.append(d_c); eg.append(g_c); ew.append(w_c)
    try:
        from scipy.optimize import linear_sum_assignment
        uraw = np.maximum(1, (craw + ALIGN - 1) // ALIGN)
        for _ in range(3):
            for c in range(NCORE):
                om = np.stack([uraw[d][perm[d]] for d in range(NCORE)
                               if d != c]).max(axis=0)
                cost = np.maximum(om[None, :, :],
                                  uraw[c][:, None, :]).sum(axis=2)
                ri, ci = linear_sum_assignment(cost)
                newp = np.zeros(wpc, np.int64)
                newp[ci] = ri
                perm[c] = newp
    except ImportError:
        pass
    cnts = np.stack([craw[c][perm[c]] for c in range(NCORE)])
    ej = []
    for c in range(NCORE):
        jmap = np.zeros(wpc, np.int64)
        jmap[perm[c]] = np.arange(wpc)
        ej.append(jmap[ew[c]])

    # unified run sizes in ALIGN-row units (max over cores, >=1 unit)
    upt = 128 // ALIGN                                # units per tile
    U = np.maximum(1, (cnts.max(axis=0) + ALIGN - 1) // ALIGN)   # [wpc, ng]

    # stream layout: (sw, g) chunks of back-to-back runs, chunk padded to
    # whole tiles; gather ops of <= max_op rows; per-op window tile spans
    ops = []
    run_u0 = np.zeros((wpc, ng), np.int64)
    upos = 0
    mt = cfg.max_op // 128
    for sw in range(cfg.nsw):
        for g in range(ng):
            chunk_u0 = upos
            bounds = []                       # (j, unit_start, unit_end)
            for jl in range(cfg.bw):
                j = sw * cfg.bw + jl
                run_u0[j, g] = upos
                bounds.append((j, upos, upos + int(U[j, g])))
                upos += int(U[j, g])
            upos = (upos + upt - 1) // upt * upt
            t0 = chunk_u0 // upt
            ct = (upos - chunk_u0) // upt
            o = 0
            while o < ct:
                nt_op = min(mt, ct - o)
                ot0, ot1 = t0 + o, t0 + o + nt_op    # op tile range
                wins = []
                for (j, a, b) in bounds:
                    ta, tb = a // upt, (b + upt - 1) // upt
                    s, e = max(ta, ot0), min(tb, ot1)
                    if s < e:
                        islast = (g == ng - 1 and e == tb)
                        wins.append((j, s - ot0, e - s, islast))
                ops.append((ot0, nt_op, g, wins))
                o += nt_op
    ntiles = upos // upt
    lay = Layout(ops=ops, ntiles=ntiles)
    rows = ntiles * 128

    # per-core idx/slot tables in the unified layout
    idx_ws, slot_tabs = [], []
    invd_arr = np.zeros((NCORE, 128, wpc), np.float32)
    rer_arr = np.zeros((NCORE, 128, wpc), np.float32)
    for c in range(NCORE):
        s_c, d_c, g_c, j_c = es[c], ed[c], eg[c], ej[c]
        key = (j_c * ng + g_c)
        order = np.argsort(key * (1 << 17) + s_c, kind="stable")
        s_c, d_c, g_c, key = (a[order] for a in (s_c, d_c, g_c, key))
        cnt = np.bincount(key, minlength=wpc * ng)
        start = np.concatenate([[0], np.cumsum(cnt)[:-1]])
        idx_arr = np.zeros(rows, np.int64)
        slot_arr = np.full(rows, PAD_SLOT, np.float32)
        for j in range(wpc):
            ph = 128.0 * (j % cfg.bw)
            for g in range(ng):
                k = j * ng + g
                n = int(cnt[k])
                p0 = int(run_u0[j, g]) * ALIGN
                if n:
                    sl = slice(start[k], start[k] + n)
                    idx_arr[p0:p0 + n] = s_c[sl] - g_c[sl] * cfg.gs
                    slot_arr[p0:p0 + n] = ph + (d_c[sl] & 127)
                    idx_arr[p0 + n:p0 + int(U[j, g]) * ALIGN] = idx_arr[p0 + n - 1]
        idx_ws.append(np.ascontiguousarray(wrap_idx(idx_arr)))
        st = slot_arr.reshape(-1, 128).T
        slot_tabs.append(np.ascontiguousarray(st.astype(np.float16)))

        lo = c * npc
        node = lo + (perm[c][:, None] * 128 + np.arange(128)[None, :])
        valid = node < N
        dv = np.zeros((wpc, 128), np.float32)
        dv[valid] = deg_full[node[valid]]
        invd_arr[c] = (1.0 / np.maximum(dv, 1.0)).T
        rv = np.zeros((wpc, 128), np.float32)
        rv[valid] = rer[node[valid]]
        rer_arr[c] = rv.T

    xT_own = np.zeros((NCORE, 2, 128, cfg.npc), ml_dtypes.bfloat16)
    for c in range(NCORE):
        lo = c * npc
        cols = (lo + perm[c][:, None] * 128 + np.arange(128)[None, :]).ravel()
        np.clip(cols, 0, cfg.nx - 1, out=cols)
        xT_own[c, 0] = xT_bf[0:128, cols]
        xT_own[c, 1] = xT_bf[128:256, cols]
    return x_gt, idx_ws, slot_tabs, invd_arr, rer_arr, xT_own, perm, lay


def build(cfg: Cfg, lay: Layout):
    nc = bass.Bass("TRN2", target_bir_lowering=False, debug=False,
                   num_devices=NCORE, dynamic_dma_scratch_size=32768,
                   num_swdge_queues=4)
    D, H = cfg.d_in, cfg.d_h
    wpc, ntiles = cfg.wpc, lay.ntiles
    GDT = FP8 if USE_FP8 else BF16
    xrows = nc.dram_tensor("xrows", [cfg.nx, D], GDT, kind="ExternalInput")
    idx = nc.dram_tensor("idx", [128, ntiles * 8], I16, kind="ExternalInput")
    slot = nc.dram_tensor("slot", [128, ntiles], F16, kind="ExternalInput")
    invd = nc.dram_tensor("invd", [128, wpc], F32, kind="ExternalInput")
    rer = nc.dram_tensor("rer", [128, wpc], F32, kind="ExternalInput")
    xto = nc.dram_tensor("xto", [2, 128, cfg.npc], BF16, kind="ExternalInput")
    wl = nc.dram_tensor("wl", [2, 128, H], BF16, kind="ExternalInput")
    wrx = nc.dram_tensor("wrx", [2, 128, H + 1], BF16, kind="ExternalInput")
    blc = nc.dram_tensor("blc", [1, H + 1], F32, kind="ExternalInput")
    blb = nc.dram_tensor("blb", [128, H], F32, kind="ExternalInput")
    wscb = nc.dram_tensor("wscb", [128, H], F32, kind="ExternalInput")
    iota7 = nc.dram_tensor("iota7", [128, cfg.bw * 128], F16,
                           kind="ExternalInput")
    out = nc.dram_tensor("out", [128, wpc], F32, kind="ExternalOutput")

    with tile.TileContext(nc) as tc:
        with (
            tc.tile_pool(name="persist", bufs=1) as pp,
            tc.tile_pool(name="gpool", bufs=10) as gpool,
            tc.tile_pool(name="mpool", bufs=10) as mpool,
            tc.tile_pool(name="wsb", bufs=4) as wsb,
            tc.tile_pool(name="apsum", bufs=(cfg.bw + 1) // 2, space="PSUM") as apsum,
            tc.tile_pool(name="tpsum", bufs=2, space="PSUM") as tpsum,
            tc.tile_pool(name="hpsum", bufs=2, space="PSUM") as hpsum,
        ):
            # ---- persistent loads -------------------------------------
            from concourse import library_config
            nc.gpsimd.load_library(library_config.mlp)
            # first gathers need only idx[0:2 ops] + slot + iota: tiny loads
            # on the sync HWDGE ring; everything else goes via the scalar
            # (ACT) HWDGE ring so it can't head-of-line block them
            idx_t = pp.tile([128, ntiles * 8], I16)
            cols = ntiles * 8
            step = ((cols + cfg.nsw - 1) // cfg.nsw + 7) // 8 * 8
            nc.scalar.dma_start(out=idx_t[:, 0:step], in_=idx[:, 0:step])
            slot_t = pp.tile([128, ntiles], F16)
            nc.scalar.dma_start(out=slot_t[:], in_=slot[:])
            iota_t = pp.tile([128, cfg.bw * 128], F16)
            nc.scalar.dma_start(out=iota_t[:], in_=iota7[:])
            for s in range(step, cols, step):
                e = min(s + step, cols)
                nc.sync.dma_start(out=idx_t[:, s:e], in_=idx[:, s:e])
            invd_t = pp.tile([128, wpc], F32)
            nc.sync.dma_start(out=invd_t[:], in_=invd[:])
            rer_t = pp.tile([128, wpc], F32)
            nc.sync.dma_start(out=rer_t[:], in_=rer[:])
            wscb_t = pp.tile([128, H], F32)
            nc.sync.dma_start(out=wscb_t[:], in_=wscb[:])
            xto_t = []
            for h in range(2):
                t = pp.tile([128, cfg.npc], BF16, tag=f"xto{h}")
                nc.sync.dma_start(out=t[:], in_=xto[h])
                xto_t.append(t)
            wl_t = []
            wrx_t = []
            for h in range(2):
                t = pp.tile([128, H], BF16, tag=f"wl{h}")
                nc.sync.dma_start(out=t[:], in_=wl[h])
                wl_t.append(t)
                t2 = pp.tile([128, H + 1], BF16, tag=f"wrx{h}")
                nc.sync.dma_start(out=t2[:], in_=wrx[h])
                wrx_t.append(t2)
            blc_t = pp.tile([1, H + 1], F32)
            nc.sync.dma_start(out=blc_t[:], in_=blc[:])
            blb_t = pp.tile([128, H], F32)
            nc.sync.dma_start(out=blb_t[:], in_=blb[:])
            ones_row = pp.tile([1, 128], F32)
            nc.vector.memset(ones_row[:], 1.0)
            out_t = pp.tile([128, wpc], F32)

            ident = pp.tile([128, 128], BF16)
            from concourse.masks import make_identity
            make_identity(nc, ident[:])

            kregs = {}
            for (_, nt, _, _) in lay.ops:
                sz = nt * 128
                if sz not in kregs:
                    kregs[sz] = nc.gpsimd.to_reg(sz)

            def epilogue(j, acc):
                aggr = wsb.tile([128, D], BF16, tag="aggr")
                nc.vector.tensor_tensor(
                    out=aggr[:], in0=acc,
                    in1=invd_t[:, j:j + 1].to_broadcast([128, D]), op=AOP.mult)
                ph = hpsum.tile([128, H + 1], F32, tag="ph")
                for h in range(2):
                    nc.tensor.matmul(
                        ph[:, 0:H + 1],
                        lhsT=xto_t[h][:, j * 128:(j + 1) * 128],
                        rhs=wrx_t[h][:], start=(h == 0), stop=False)
                if USE_BLC:
                    nc.tensor.matmul(ph[:, 0:H + 1], lhsT=ones_row[:],
                                     rhs=blc_t[:], start=False, stop=False)
                for h in range(2):
                    pt = tpsum.tile([128, 128], BF16, tag="pt")
                    nc.tensor.transpose(out=pt[:],
                                        in_=aggr[:, h * 128:(h + 1) * 128],
                                        identity=ident[:])
                    aggrT = wsb.tile([128, 128], BF16, tag=f"aggrT{h}")
                    if USE_ACTCOPY:
                        nc.scalar.activation(out=aggrT[:], in_=pt[:],
                                             func=ACT.Copy)
                    else:
                        nc.vector.tensor_copy(out=aggrT[:], in_=pt[:])
                    nc.tensor.matmul(ph[:, 0:H], lhsT=aggrT[:], rhs=wl_t[h][:],
                                     start=False, stop=(h == 1))
                if USE_BLC:
                    hin = ph[:, 0:H]
                else:
                    hpre = wsb.tile([128, H], F32, tag="hpre")
                    nc.vector.tensor_add(out=hpre[:], in0=ph[:, 0:H],
                                         in1=blb_t[:])
                    hin = hpre[:]
                hrelu = wsb.tile([128, H], F32, tag="hrelu")
                nc.scalar.activation(out=hrelu[:], in_=hin, func=ACT.Relu)
                hw = wsb.tile([128, H], F32, tag="hw")
                if USE_TTR:
                    nc.vector.tensor_tensor_reduce(
                        out=hw[:], in0=hrelu[:], in1=wscb_t[:], scale=1.0,
                        scalar=ph[:, H:H + 1], op0=AOP.mult, op1=AOP.add,
                        accum_out=out_t[:, j:j + 1])
                else:
                    nc.vector.tensor_tensor(out=hw[:], in0=hrelu[:],
                                            in1=wscb_t[:], op=AOP.mult)
                    gdot = wsb.tile([128, 1], F32, tag="gdot")
                    nc.vector.reduce_sum(out=gdot[:], in_=hw[:],
                                         axis=mybir.AxisListType.X)
                    nc.vector.tensor_add(out=out_t[:, j:j + 1], in0=gdot[:],
                                         in1=ph[:, H:H + 1])

            # ---- main loop --------------------------------------------
            accs = {}
            started = set()
            for opi, (t0, nt, g, wins) in enumerate(lay.ops):
                sz = nt * 128
                gbf = gpool.tile([128, cfg.max_op // 128, D], GDT, tag="gb")
                gb = gbf[:, 0:nt, :]
                nc.gpsimd.dma_gather(
                    out_ap=gb[:], in_ap=xrows[g * cfg.gs:(g + 1) * cfg.gs, :],
                    idxs_ap=idx_t[:, t0 * 8:t0 * 8 + sz // 16],
                    num_idxs=sz, num_idxs_reg=kregs[sz],
                    elem_size=D, queue_num=opi % 4)
                for (j, tloc, ntw, islast) in wins:
                    jl = j % cfg.bw
                    mkf = mpool.tile([128, cfg.max_op // 128, 128], GDT,
                                     tag="mk")
                    mk = mkf[:, 0:ntw, :]
                    nc.vector.tensor_tensor(
                        out=mk[:],
                        in0=slot_t[:, t0 + tloc:t0 + tloc + ntw].unsqueeze(2)
                            .to_broadcast([128, ntw, 128]),
                        in1=iota_t[:, jl * 128:(jl + 1) * 128].unsqueeze(1)
                            .to_broadcast([128, ntw, 128]),
                        op=AOP.is_equal)
                    sw = j // cfg.bw
                    pkey = (sw, jl // 2)
                    sub = jl % 2
                    if pkey not in accs:
                        accs[pkey] = apsum.tile([128, 2 * D], F32, tag="acc",
                                                name=f"accp{pkey[1]}")
                    acc = accs[pkey][:, sub * D:(sub + 1) * D]
                    st = pkey not in started
                    started.add(pkey)
                    i = 0
                    while i < ntw:
                        pair = USE_FP8 and i + 1 < ntw
                        lastm = (i + (2 if pair else 1) >= ntw) and islast
                        if pair:
                            nc.tensor.matmul(
                                acc, lhsT=mk[:, i:i + 2, :],
                                rhs=gb[:, tloc + i:tloc + i + 2, :],
                                start=st and i == 0, stop=lastm,
                                perf_mode=mybir.MatmulPerfMode.DoubleRow)
                            i += 2
                        else:
                            nc.tensor.matmul(
                                acc, lhsT=mk[:, i, :], rhs=gb[:, tloc + i, :],
                                start=st and i == 0, stop=lastm)
                            i += 1
                    if islast:
                        epilogue(j, acc)

            nc.vector.tensor_add(out=out_t[:], in0=out_t[:], in1=rer_t[:])
            nc.sync.dma_start(out=out[:], in_=out_t[:])

    return finish(nc)


def kernel_impl(x, edge_index, reranker_scores, W_l, b_l, W_r, W_res, b_res,
                w_score, b_score, alpha, trace=False):
    N = int(x.shape[0])
    cfg = Cfg(nsw=14, bw=7, nx=100096, gs=25024)
    assert cfg.npc * NCORE >= N

    (x_gt, idx_ws, slot_tabs, invd_arr, rer_arr, xT_own, perm,
     lay) = preprocess(x, edge_index, reranker_scores, cfg)

    # host-folded small-weight math
    W_l = np.asarray(W_l, np.float64)
    W_r = np.asarray(W_r, np.float64)
    W_res = np.asarray(W_res, np.float64)
    w_score = np.asarray(w_score, np.float64)
    a = float(1.0 / (1.0 + np.exp(-float(np.asarray(alpha)))))
    oma = 1.0 - a
    u = W_res.T @ w_score                      # [256]
    cterm = float(np.asarray(b_res, np.float64) @ w_score
                  + float(np.asarray(b_score)))
    wrx_host = np.zeros((2, 128, cfg.d_h + 1), np.float32)
    wl_host = np.zeros((2, 128, cfg.d_h), np.float32)
    for h in range(2):
        wrx_host[h, :, 0:cfg.d_h] = W_r.T[h * 128:(h + 1) * 128, :]
        wrx_host[h, :, cfg.d_h] = oma * u[h * 128:(h + 1) * 128]
        wl_host[h] = W_l.T[h * 128:(h + 1) * 128, :]
    blc_host = np.zeros((1, cfg.d_h + 1), np.float32)
    blc_host[0, 0:cfg.d_h] = np.asarray(b_l, np.float32)
    blc_host[0, cfg.d_h] = oma * cterm
    band = np.arange(cfg.bw * 128, dtype=np.float32).astype(np.float16)
    iota_host = np.ascontiguousarray(
        np.broadcast_to(band, (128, cfg.bw * 128)))

    common = {
        "xrows": x_gt,
        "wl": wl_host.astype(ml_dtypes.bfloat16),
        "wrx": wrx_host.astype(ml_dtypes.bfloat16),
        "blc": blc_host,
        "blb": np.ascontiguousarray(np.broadcast_to(
            np.asarray(b_l, np.float32), (128, cfg.d_h))),
        "wscb": np.ascontiguousarray(np.broadcast_to(
            (oma * w_score).astype(np.float32), (128, cfg.d_h))),
        "iota7": iota_host,
    }
    rer_const = 0.0 if USE_BLC else oma * cterm
    nc = build(cfg, lay)
    in_maps = []
    for c_i in range(NCORE):
        im = dict(common)
        im["idx"] = idx_ws[c_i]
        im["slot"] = slot_tabs[c_i]
        im["invd"] = np.ascontiguousarray(invd_arr[c_i])
        im["rer"] = np.ascontiguousarray(
            (rer_arr[c_i] * a + rer_const).astype(np.float32))
        im["xto"] = np.ascontiguousarray(xT_own[c_i])
        in_maps.append(im)

    # The very first execution of a freshly-compiled NEFF has been
    # observed (rarely) to fault or return corrupted data; do an untraced
    # warmup execution first, then the real run, with one retry on error.
    res = None
    for attempt in range(3):
        try:
            res = run_bass_kernel_spmd(nc, in_maps,
                                       core_ids=list(range(NCORE)),
                                       trace=trace)
            break
        except Exception:
            if attempt == 2:
                raise
            import time
            time.sleep(5)
    full = np.zeros(N, np.float32)
    for c_i in range(NCORE):
        oc = np.asarray(res.results[c_i]["out"], np.float32)  # [128, wpc]
        lo = c_i * cfg.npc
        node = lo + (perm[c_i][:, None] * 128 + np.arange(128)[None, :])
        valid = node < N
        full[node[valid]] = oc.T[valid]
    return (full, res) if trace else full


def kernel(**inputs):
    out = kernel_impl(
        np.asarray(inputs["x"]),
        np.asarray(inputs["edge_index"]),
        np.asarray(inputs["reranker_scores"]),
        np.asarray(inputs["W_l"]),
        np.asarray(inputs["b_l"]),
        np.asarray(inputs["W_r"]),
        np.asarray(inputs["W_res"]),
        np.asarray(inputs["b_res"]),
        np.asarray(inputs["w_score"]),
        np.asarray(inputs["b_score"]),
        np.asarray(inputs["alpha"]),
    )
    return out.astype(np.float32)


# revision 37
# speedup vs baseline: 1.0839x; 1.0839x over previous
"""ContrastivePretrainedSAGE Trainium2 kernel v4 (8-core SPMD).

Design: nodes sharded by id range (12544 slots/core = 98 windows of 128).
Edges routed to the dst-owning core. Within each core, windows are
PERMUTED (sorted by incoming-edge count, descending) so that window-slot
j has a similar count on every core; the single SPMD program sizes each
(slot, src-group) run as max-over-cores ceil(cnt/16) 16-row units
(~218k gathered rows/core vs 250.9k for fixed 5-tile runs). Runs pack
back-to-back in the (superwindow, group) chunk stream with no alignment
constraints: the slot table PHASE-ENCODES the target window
(slot16 = 128*jl + dst_slot, fp16-exact, jl unique within a chunk), so a
window's mask - built by one is_equal against that window's iota band -
automatically zeroes every row belonging to other windows or padding.
Every mask matmul is then a full K=128, base-partition-0 matmul (the
only PE config that runs reliably), regardless of where runs start/end.

Source features are fetched with InstDMAGatherAnt from an fp8(e4m3) copy
of x (256B rows), 1024 rows per op, FOUR SWDGE queues so descriptor
generation runs concurrently (measured: 1q 8.8 ns/idx, 2q 4.6, 4q
3.2-3.6 - the Q7 descriptor generator is this kernel's bottleneck).
Mask matmuls accumulate into a [128,256] f32 PSUM region per window
(2 windows/bank); adjacent tile pairs of the same window fuse into one
fp8 DoubleRow matmul (K=256/pass, 2x PE rate).

Everything derivable from the small weights is folded on the host
(u=W_res.T@w_score, c=b_res.w_score+b_score, a=sigmoid(alpha), with
(1-a) pre-multiplied into w_score/u/c and a into reranker_scores), so
the per-window epilogue is only:
  DVE:  aggr = psum * invd          (1 instr)
  PE:   ph = x@[W_r.T | u'] + ones@[b_l | c'] + aggr@W_l.T  (via 2
        transposes + 5 matmuls)
  ACT:  hrelu = relu(ph[:,0:128]), 2 PSUM->SBUF transpose copies
  DVE:  out[:,j] = reduce_add(hrelu * ws', init=ph[:,128])  (1 instr)
and one final out += a*rer over all windows. deg/invd are host-side
(index-space bincount).
"""
from dataclasses import dataclass

import numpy as np
import ml_dtypes

import concourse.bass as bass
import concourse.mybir as mybir
import concourse.tile as tile
from concourse.bass_utils import run_bass_kernel_spmd

F32 = mybir.dt.float32
F16 = mybir.dt.float16
BF16 = mybir.dt.bfloat16
FP8 = mybir.dt.float8e4
I16 = mybir.dt.int16
AOP = mybir.AluOpType
ACT = mybir.ActivationFunctionType
NCORE = 8
PAD_SLOT = 1023.0
NP_FP8 = ml_dtypes.float8_e4m3
USE_FP8 = True
USE_TTR = False      # fuse h*ws + reduce + xu via tensor_tensor_reduce
USE_ACTCOPY = False  # PSUM->SBUF transpose copies on ACT instead of DVE
USE_BLC = True      # add [b_l | c] via ones-row matmul instead of DVE
ALIGN = 16          # run-length granularity in rows


def split_sync_waits(nc) -> int:
    n_split = 0
    for f in nc.m.functions:
        for bb in f.blocks:
            out = []
            changed = False
            for ins in bb.instructions:
                si = ins.sync_info
                waits = list(si.on_wait) if si is not None and si.on_wait else []
                if len(waits) > 1:
                    for g, w in enumerate(waits[:-1]):
                        nop = mybir.InstNoOp(name=f"{ins.name}-waitsplit-{g}")
                        nop.engine = ins.engine
                        nop.sync_info = mybir.SyncInfo(on_wait=[w], on_update=[])
                        out.append(nop)
                    si.on_wait = waits[-1:]
                    changed = True
                    n_split += 1
                out.append(ins)
            if changed:
                bb.instructions.clear()
                for i in out:
                    bb.instructions.append(i)
    return n_split


def finish(nc):
    split_sync_waits(nc)
    import bass_rust
    from concourse.library_config import all_libraries, standard
    m = {}
    for lib in all_libraries:
        for it in lib.instructions:
            m[it] = m.get(it, 0) | (1 << lib.index)
    bass_rust.insert_library_loads(nc, m, len(all_libraries), standard.index)
    mybir.codegen_inst_isa_subclasses(nc)
    return nc


@dataclass
class Cfg:
    nsw: int          # superwindows per core
    bw: int           # windows (slots) per superwindow
    nx: int           # padded gather-table rows
    gs: int           # group size (rows per source group, <= 32768)
    ngroups: int = 4
    d_in: int = 256
    d_h: int = 128
    max_op: int = 1024

    @property
    def wpc(self):
        return self.nsw * self.bw

    @property
    def npc(self):
        return self.wpc * 128


@dataclass
class Layout:
    """Unified (all-core) tile-stream layout, host-computed.

    ops: list of gather ops (t0, nt, g, wins) where wins is the list of
         (j, tloc, ntw, islast): window-slot j covers op-local tiles
         [tloc, tloc+ntw); islast marks the window's final op -> its
         epilogue runs after those matmuls.
    ntiles: stream length in tiles
    """
    ops: list
    ntiles: int


def wrap_idx(idx: np.ndarray) -> np.ndarray:
    """[L] -> [128, L/16] int16 wrapped (i at [i%16, i//16]), replicated 8x."""
    L = len(idx)
    assert L % 16 == 0
    block = np.zeros((16, L // 16), np.int16)
    block[np.arange(L) % 16, np.arange(L) // 16] = idx.astype(np.int16)
    return np.tile(block, (8, 1))


def preprocess(x, edge_index, reranker_scores, cfg: Cfg):
    """Index-space edge routing + pure layout prep of per-core inputs."""
    N = x.shape[0]
    src = np.asarray(edge_index[0], dtype=np.int64)
    dst = np.asarray(edge_index[1], dtype=np.int64)
    rer = np.asarray(reranker_scores, dtype=np.float32)

    x_pad = np.zeros((cfg.nx, cfg.d_in), np.float32)
    x_pad[:N] = np.asarray(x, dtype=np.float32)
    x_gt = np.ascontiguousarray(
        x_pad.astype(NP_FP8 if USE_FP8 else ml_dtypes.bfloat16))
    xT_bf = np.ascontiguousarray(x_pad.astype(ml_dtypes.bfloat16).T)

    npc, wpc, ng = cfg.npc, cfg.wpc, cfg.ngroups
    g_of = src // cfg.gs
    deg_full = np.bincount(dst, minlength=N)

    # per-core routing + window permutation (sorted by count desc)
    perm = np.zeros((NCORE, wpc), np.int64)       # slot j -> orig window
    cnts = np.zeros((NCORE, wpc, ng), np.int64)
    es, eg, ed, ej = [], [], [], []
    for c in range(NCORE):
        lo = c * npc
        m = (dst >= lo) & (dst < lo + npc)
        s_c = src[m]
        d_c = dst[m] - lo
        g_c = g_of[m]
        w_c = d_c >> 7
        wtot = np.bincount(w_c, minlength=wpc)
        order_w = np.argsort(-wtot, kind="stable")
        perm[c] = order_w
        jmap = np.zeros(wpc, np.int64)
        jmap[order_w] = np.arange(wpc)
        j_c = jmap[w_c]
        cnts[c] = np.bincount(j_c * ng + g_c, minlength=wpc * ng).reshape(wpc, ng)
        es.append(s_c); ed.append(d_c); eg.append(g_c); ej.append(j_c)

    # unified run sizes in ALIGN-row units (max over cores, >=1 unit)
    upt = 128 // ALIGN                                # units per tile
    U = np.maximum(1, (cnts.max(axis=0) + ALIGN - 1) // ALIGN)   # [wpc, ng]

    # stream layout: (sw, g) chunks of back-to-back runs, chunk padded to
    # whole tiles; gather ops of <= max_op rows; per-op window tile spans
    ops = []
    run_u0 = np.zeros((wpc, ng), np.int64)
    upos = 0
    mt = cfg.max_op // 128
    for sw in range(cfg.nsw):
        for g in range(ng):
            chunk_u0 = upos
            bounds = []                       # (j, unit_start, unit_end)
            for jl in range(cfg.bw):
                j = sw * cfg.bw + jl
                run_u0[j, g] = upos
                bounds.append((j, upos, upos + int(U[j, g])))
                upos += int(U[j, g])
            upos = (upos + upt - 1) // upt * upt
            t0 = chunk_u0 // upt
            ct = (upos - chunk_u0) // upt
            o = 0
            while o < ct:
                nt_op = min(mt, ct - o)
                ot0, ot1 = t0 + o, t0 + o + nt_op    # op tile range
                wins = []
                for (j, a, b) in bounds:
                    ta, tb = a // upt, (b + upt - 1) // upt
                    s, e = max(ta, ot0), min(tb, ot1)
                    if s < e:
                        islast = (g == ng - 1 and e == tb)
                        wins.append((j, s - ot0, e - s, islast))
                ops.append((ot0, nt_op, g, wins))
                o += nt_op
    ntiles = upos // upt
    lay = Layout(ops=ops, ntiles=ntiles)
    rows = ntiles * 128

    # per-core idx/slot tables in the unified layout
    idx_ws, slot_tabs = [], []
    invd_arr = np.zeros((NCORE, 128, wpc), np.float32)
    rer_arr = np.zeros((NCORE, 128, wpc), np.float32)
    for c in range(NCORE):
        s_c, d_c, g_c, j_c = es[c], ed[c], eg[c], ej[c]
        key = (j_c * ng + g_c)
        order = np.argsort(key * (1 << 17) + s_c, kind="stable")
        s_c, d_c, g_c, key = (a[order] for a in (s_c, d_c, g_c, key))
        cnt = np.bincount(key, minlength=wpc * ng)
        start = np.concatenate([[0], np.cumsum(cnt)[:-1]])
        idx_arr = np.zeros(rows, np.int64)
        slot_arr = np.full(rows, PAD_SLOT, np.float32)
        for j in range(wpc):
            ph = 128.0 * (j % cfg.bw)
            for g in range(ng):
                k = j * ng + g
                n = int(cnt[k])
                p0 = int(run_u0[j, g]) * ALIGN
                if n:
                    sl = slice(start[k], start[k] + n)
                    idx_arr[p0:p0 + n] = s_c[sl] - g_c[sl] * cfg.gs
                    slot_arr[p0:p0 + n] = ph + (d_c[sl] & 127)
                    idx_arr[p0 + n:p0 + int(U[j, g]) * ALIGN] = idx_arr[p0 + n - 1]
        idx_ws.append(np.ascontiguousarray(wrap_idx(idx_arr)))
        st = slot_arr.reshape(-1, 128).T
        slot_tabs.append(np.ascontiguousarray(st.astype(np.float16)))

        lo = c * npc
        node = lo + (perm[c][:, None] * 128 + np.arange(128)[None, :])
        valid = node < N
        dv = np.zeros((wpc, 128), np.float32)
        dv[valid] = deg_full[node[valid]]
        invd_arr[c] = (1.0 / np.maximum(dv, 1.0)).T
        rv = np.zeros((wpc, 128), np.float32)
        rv[valid] = rer[node[valid]]
        rer_arr[c] = rv.T

    xT_own = np.zeros((NCORE, 2, 128, cfg.npc), ml_dtypes.bfloat16)
    for c in range(NCORE):
        lo = c * npc
        cols = (lo + perm[c][:, None] * 128 + np.arange(128)[None, :]).ravel()
        np.clip(cols, 0, cfg.nx - 1, out=cols)
        xT_own[c, 0] = xT_bf[0:128, cols]
        xT_own[c, 1] = xT_bf[128:256, cols]
    return x_gt, idx_ws, slot_tabs, invd_arr, rer_arr, xT_own, perm, lay


def build(cfg: Cfg, lay: Layout):
    nc = bass.Bass("TRN2", target_bir_lowering=False, debug=False,
                   num_devices=NCORE, dynamic_dma_scratch_size=32768,
                   num_swdge_queues=4)
    D, H = cfg.d_in, cfg.d_h
    wpc, ntiles = cfg.wpc, lay.ntiles
    GDT = FP8 if USE_FP8 else BF16
    xrows = nc.dram_tensor("xrows", [cfg.nx, D], GDT, kind="ExternalInput")
    idx = nc.dram_tensor("idx", [128, ntiles * 8], I16, kind="ExternalInput")
    slot = nc.dram_tensor("slot", [128, ntiles], F16, kind="ExternalInput")
    invd = nc.dram_tensor("invd", [128, wpc], F32, kind="ExternalInput")
    rer = nc.dram_tensor("rer", [128, wpc], F32, kind="ExternalInput")
    xto = nc.dram_tensor("xto", [2, 128, cfg.npc], BF16, kind="ExternalInput")
    wl = nc.dram_tensor("wl", [2, 128, H], BF16, kind="ExternalInput")
    wrx = nc.dram_tensor("wrx", [2, 128, H + 1], BF16, kind="ExternalInput")
    blc = nc.dram_tensor("blc", [1, H + 1], F32, kind="ExternalInput")
    blb = nc.dram_tensor("blb", [128, H], F32, kind="ExternalInput")
    wscb = nc.dram_tensor("wscb", [128, H], F32, kind="ExternalInput")
    iota7 = nc.dram_tensor("iota7", [128, cfg.bw * 128], F16,
                           kind="ExternalInput")
    out = nc.dram_tensor("out", [128, wpc], F32, kind="ExternalOutput")

    with tile.TileContext(nc) as tc:
        with (
            tc.tile_pool(name="persist", bufs=1) as pp,
            tc.tile_pool(name="gpool", bufs=10) as gpool,
            tc.tile_pool(name="mpool", bufs=10) as mpool,
            tc.tile_pool(name="wsb", bufs=4) as wsb,
            tc.tile_pool(name="apsum", bufs=(cfg.bw + 1) // 2, space="PSUM") as apsum,
            tc.tile_pool(name="tpsum", bufs=2, space="PSUM") as tpsum,
            tc.tile_pool(name="hpsum", bufs=2, space="PSUM") as hpsum,
        ):
            # ---- persistent loads -------------------------------------
            from concourse import library_config
            nc.gpsimd.load_library(library_config.mlp)
            # first gathers need only idx[0:2 ops] + slot + iota: tiny loads
            # on the sync HWDGE ring; everything else goes via the scalar
            # (ACT) HWDGE ring so it can't head-of-line block them
            idx_t = pp.tile([128, ntiles * 8], I16)
            cols = ntiles * 8
            step = ((cols + cfg.nsw - 1) // cfg.nsw + 7) // 8 * 8
            nc.scalar.dma_start(out=idx_t[:, 0:step], in_=idx[:, 0:step])
            slot_t = pp.tile([128, ntiles], F16)
            nc.scalar.dma_start(out=slot_t[:], in_=slot[:])
            iota_t = pp.tile([128, cfg.bw * 128], F16)
            nc.scalar.dma_start(out=iota_t[:], in_=iota7[:])
            for s in range(step, cols, step):
                e = min(s + step, cols)
                nc.sync.dma_start(out=idx_t[:, s:e], in_=idx[:, s:e])
            invd_t = pp.tile([128, wpc], F32)
            nc.sync.dma_start(out=invd_t[:], in_=invd[:])
            rer_t = pp.tile([128, wpc], F32)
            nc.sync.dma_start(out=rer_t[:], in_=rer[:])
            wscb_t = pp.tile([128, H], F32)
            nc.sync.dma_start(out=wscb_t[:], in_=wscb[:])
            xto_t = []
            for h in range(2):
                t = pp.tile([128, cfg.npc], BF16, tag=f"xto{h}")
                nc.sync.dma_start(out=t[:], in_=xto[h])
                xto_t.append(t)
            wl_t = []
            wrx_t = []
            for h in range(2):
                t = pp.tile([128, H], BF16, tag=f"wl{h}")
                nc.sync.dma_start(out=t[:], in_=wl[h])
                wl_t.append(t)
                t2 = pp.tile([128, H + 1], BF16, tag=f"wrx{h}")
                nc.sync.dma_start(out=t2[:], in_=wrx[h])
                wrx_t.append(t2)
            blc_t = pp.tile([1, H + 1], F32)
            nc.sync.dma_start(out=blc_t[:], in_=blc[:])
            blb_t = pp.tile([128, H], F32)
            nc.sync.dma_start(out=blb_t[:], in_=blb[:])
            ones_row = pp.tile([1, 128], F32)
            nc.vector.memset(ones_row[:], 1.0)
            out_t = pp.tile([128, wpc], F32)

            ident = pp.tile([128, 128], BF16)
            from concourse.masks import make_identity
            make_identity(nc, ident[:])

            kregs = {}
            for (_, nt, _, _) in lay.ops:
                sz = nt * 128
                if sz not in kregs:
                    kregs[sz] = nc.gpsimd.to_reg(sz)

            def epilogue(j, acc):
                aggr = wsb.tile([128, D], BF16, tag="aggr")
                nc.vector.tensor_tensor(
                    out=aggr[:], in0=acc,
                    in1=invd_t[:, j:j + 1].to_broadcast([128, D]), op=AOP.mult)
                ph = hpsum.tile([128, H + 1], F32, tag="ph")
                for h in range(2):
                    nc.tensor.matmul(
                        ph[:, 0:H + 1],
                        lhsT=xto_t[h][:, j * 128:(j + 1) * 128],
                        rhs=wrx_t[h][:], start=(h == 0), stop=False)
                if USE_BLC:
                    nc.tensor.matmul(ph[:, 0:H + 1], lhsT=ones_row[:],
                                     rhs=blc_t[:], start=False, stop=False)
                for h in range(2):
                    pt = tpsum.tile([128, 128], BF16, tag="pt")
                    nc.tensor.transpose(out=pt[:],
                                        in_=aggr[:, h * 128:(h + 1) * 128],
                                        identity=ident[:])
                    aggrT = wsb.tile([128, 128], BF16, tag=f"aggrT{h}")
                    if USE_ACTCOPY:
                        nc.scalar.activation(out=aggrT[:], in_=pt[:],
                                             func=ACT.Copy)
                    else:
                        nc.vector.tensor_copy(out=aggrT[:], in_=pt[:])
                    nc.tensor.matmul(ph[:, 0:H], lhsT=aggrT[:], rhs=wl_t[h][:],
                                     start=False, stop=(h == 1))
                if USE_BLC:
                    hin = ph[:, 0:H]
                else:
                    hpre = wsb.tile([128, H], F32, tag="hpre")
                    nc.vector.tensor_add(out=hpre[:], in0=ph[:, 0:H],
                                         in1=blb_t[:])
                    hin = hpre[:]
                hrelu = wsb.tile([128, H], F32, tag="hrelu")
                nc.scalar.activation(out=hrelu[:], in_=hin, func=ACT.Relu)
                hw = wsb.tile([128, H], F32, tag="hw")
                if USE_TTR:
                    nc.vector.tensor_tensor_reduce(
                        out=hw[:], in0=hrelu[:], in1=wscb_t[:], scale=1.0,
                        scalar=ph[:, H:H + 1], op0=AOP.mult, op1=AOP.add,
                        accum_out=out_t[:, j:j + 1])
                else:
                    nc.vector.tensor_tensor(out=hw[:], in0=hrelu[:],
                                            in1=wscb_t[:], op=AOP.mult)
                    gdot = wsb.tile([128, 1], F32, tag="gdot")
                    nc.vector.reduce_sum(out=gdot[:], in_=hw[:],
                                         axis=mybir.AxisListType.X)
                    nc.vector.tensor_add(out=out_t[:, j:j + 1], in0=gdot[:],
                                         in1=ph[:, H:H + 1])

            # ---- main loop --------------------------------------------
            accs = {}
            started = set()
            for opi, (t0, nt, g, wins) in enumerate(lay.ops):
                sz = nt * 128
                gbf = gpool.tile([128, cfg.max_op // 128, D], GDT, tag="gb")
                gb = gbf[:, 0:nt, :]
                nc.gpsimd.dma_gather(
                    out_ap=gb[:], in_ap=xrows[g * cfg.gs:(g + 1) * cfg.gs, :],
                    idxs_ap=idx_t[:, t0 * 8:t0 * 8 + sz // 16],
                    num_idxs=sz, num_idxs_reg=kregs[sz],
                    elem_size=D, queue_num=opi % 4)
                for (j, tloc, ntw, islast) in wins:
                    jl = j % cfg.bw
                    mkf = mpool.tile([128, cfg.max_op // 128, 128], GDT,
                                     tag="mk")
                    mk = mkf[:, 0:ntw, :]
                    nc.vector.tensor_tensor(
                        out=mk[:],
                        in0=slot_t[:, t0 + tloc:t0 + tloc + ntw].unsqueeze(2)
                            .to_broadcast([128, ntw, 128]),
                        in1=iota_t[:, jl * 128:(jl + 1) * 128].unsqueeze(1)
                            .to_broadcast([128, ntw, 128]),
                        op=AOP.is_equal)
                    sw = j // cfg.bw
                    pkey = (sw, jl // 2)
                    sub = jl % 2
                    if pkey not in accs:
                        accs[pkey] = apsum.tile([128, 2 * D], F32, tag="acc",
                                                name=f"accp{pkey[1]}")
                    acc = accs[pkey][:, sub * D:(sub + 1) * D]
                    st = pkey not in started
                    started.add(pkey)
                    i = 0
                    while i < ntw:
                        pair = USE_FP8 and i + 1 < ntw
                        lastm = (i + (2 if pair else 1) >= ntw) and islast
                        if pair:
                            nc.tensor.matmul(
                                acc, lhsT=mk[:, i:i + 2, :],
                                rhs=gb[:, tloc + i:tloc + i + 2, :],
                                start=st and i == 0, stop=lastm,
                                perf_mode=mybir.MatmulPerfMode.DoubleRow)
                            i += 2
                        else:
                            nc.tensor.matmul(
                                acc, lhsT=mk[:, i, :], rhs=gb[:, tloc + i, :],
                                start=st and i == 0, stop=lastm)
                            i += 1
                    if islast:
                        epilogue(j, acc)

            nc.vector.tensor_add(out=out_t[:], in0=out_t[:], in1=rer_t[:])
            nc.sync.dma_start(out=out[:], in_=out_t[:])

    return finish(nc)


def kernel_impl(x, edge_index, reranker_scores, W_l, b_l, W_r, W_res, b_res,
                w_score, b_score, alpha, trace=False):
    N = int(x.shape[0])
    cfg = Cfg(nsw=14, bw=7, nx=100096, gs=25024)
    assert cfg.npc * NCORE >= N

    (x_gt, idx_ws, slot_tabs, invd_arr, rer_arr, xT_own, perm,
     lay) = preprocess(x, edge_index, reranker_scores, cfg)

    # host-folded small-weight math
    W_l = np.asarray(W_l, np.float64)
    W_r = np.asarray(W_r, np.float64)
    W_res = np.asarray(W_res, np.float64)
    w_score = np.asarray(w_score, np.float64)
    a = float(1.0 / (1.0 + np.exp(-float(np.asarray(alpha)))))
    oma = 1.0 - a
    u = W_res.T @ w_score                      # [256]
    cterm = float(np.asarray(b_res, np.float64) @ w_score
                  + float(np.asarray(b_score)))
    wrx_host = np.zeros((2, 128, cfg.d_h + 1), np.float32)
    wl_host = np.zeros((2, 128, cfg.d_h), np.float32)
    for h in range(2):
        wrx_host[h, :, 0:cfg.d_h] = W_r.T[h * 128:(h + 1) * 128, :]
        wrx_host[h, :, cfg.d_h] = oma * u[h * 128:(h + 1) * 128]
        wl_host[h] = W_l.T[h * 128:(h + 1) * 128, :]
    blc_host = np.zeros((1, cfg.d_h + 1), np.float32)
    blc_host[0, 0:cfg.d_h] = np.asarray(b_l, np.float32)
    blc_host[0, cfg.d_h] = oma * cterm
    band = np.arange(cfg.bw * 128, dtype=np.float32).astype(np.float16)
    iota_host = np.ascontiguousarray(
        np.broadcast_to(band, (128, cfg.bw * 128)))

    common = {
        "xrows": x_gt,
        "wl": wl_host.astype(ml_dtypes.bfloat16),
        "wrx": wrx_host.astype(ml_dtypes.bfloat16),
        "blc": blc_host,
        "blb": np.ascontiguousarray(np.broadcast_to(
            np.asarray(b_l, np.float32), (128, cfg.d_h))),
        "wscb": np.ascontiguousarray(np.broadcast_to(
            (oma * w_score).astype(np.float32), (128, cfg.d_h))),
        "iota7": iota_host,
    }
    rer_const = 0.0 if USE_BLC else oma * cterm
    nc = build(cfg, lay)
    in_maps = []
    for c_i in range(NCORE):
        im = dict(common)
        im["idx"] = idx_ws[c_i]
        im["slot"] = slot_tabs[c_i]
        im["invd"] = np.ascontiguousarray(invd_arr[c_i])
        im["rer"] = np.ascontiguousarray(
            (rer_arr[c_i] * a + rer_const).astype(np.float32))
        im["xto"] = np.ascontiguousarray(xT_own[c_i])
        in_maps.append(im)

    # The very first execution of a freshly-compiled NEFF has been
    # observed (rarely) to fault or return corrupted data; do an untraced
    # warmup execution first, then the real run, with one retry on error.
    res = None
    for attempt in range(3):
        try:
            res = run_bass_kernel_spmd(nc, in_maps,
                                       core_ids=list(range(NCORE)),
                                       trace=trace)
            break
        except Exception:
            if attempt == 2:
                raise
            import time
            time.sleep(5)
    full = np.zeros(N, np.float32)
    for c_i in range(NCORE):
        oc = np.asarray(res.results[c_i]["out"], np.float32)  # [128, wpc]
        lo = c_i * cfg.npc
        node = lo + (perm[c_i][:, None] * 128 + np.arange(128)[None, :])
        valid = node < N
        full[node[valid]] = oc.T[valid]
    return (full, res) if trace else full


def kernel(**inputs):
    out = kernel_impl(
        np.asarray(inputs["x"]),
        np.asarray(inputs["edge_index"]),
        np.asarray(inputs["reranker_scores"]),
        np.asarray(inputs["W_l"]),
        np.asarray(inputs["b_l"]),
        np.asarray(inputs["W_r"]),
        np.asarray(inputs["W_res"]),
        np.asarray(inputs["b_res"]),
        np.asarray(inputs["w_score"]),
        np.asarray(inputs["b_score"]),
        np.asarray(inputs["alpha"]),
    )
    return out.astype(np.float32)
